# revision 25
# baseline (speedup 1.0000x reference)
"""Trainium2 Bass kernel for nn_MultiHeadAttention (S=2048, B=4, D=1024, H=16).

Sharding: 8 cores = 4 batches x 2 head-groups (8 heads each).
Each core computes, for its (batch b, head-group g):
  Q^T/K^T = Wslice @ x^T (dim-major), V = x @ Wslice^T (seq-major, + ones col)
  S^T = K_h @ Q_h^T  (keys on partitions), P^T = exp(S^T/8)  (ACT, bf16)
  ctx^T+sums = [V_h|1]^T @ P^T  (PSUM accum over key chunks)
  ctx = ctx_unnorm * (1/sums)   (DVE recip + DMA broadcast)
  out^T_partial = Wo_slice^T.T @ ctx^T
Host: out[:, b, :] = (partial[2b] + partial[2b+1]).T + bo
"""

import sys

for _p in ("/opt/trn_rl_repo", "/opt/pypackages"):
    if _p not in sys.path:
        sys.path.append(_p)

import numpy as np
import ml_dtypes

import concourse.bacc as bacc
import concourse.tile as tile
from concourse import mybir
from concourse.bass_utils import run_bass_kernel_spmd

BF16 = ml_dtypes.bfloat16
FP32 = np.float32

D = 1024          # model dim
H_CORE = 8        # heads per core
DK = 64           # head dim
DH = H_CORE * DK  # 512 per-core head dims
N_CORES = 8

bf = mybir.dt.bfloat16
f32 = mybir.dt.float32


def build_program(S=2048, has_bias=True):
    """Build the per-core SPMD Bass program. Returns (nc, names)."""
    assert S % 512 == 0
    NSC = S // 128    # seq/key chunks of 128
    NQT = S // 512    # q tiles of 512
    NKD = D // 128    # 8 model-dim contraction chunks
    NKO = DH // 128   # 4 ctx contraction chunks
    NMO = D // 128    # 8 out-dim chunks
    NHP = H_CORE // 2 # 4 head pairs

    nc = bacc.Bacc(None, target_bir_lowering=False)

    xq_d = nc.dram_tensor("xq", [D, S], bf, kind="ExternalInput")
    xk_d = nc.dram_tensor("xk", [D, S], bf, kind="ExternalInput")
    xv_d = nc.dram_tensor("xv", [D, S], bf, kind="ExternalInput")
    # weights pre-tiled on host to [128, nchunks*cols] partition-major
    wq_d = nc.dram_tensor("wq", [128, NKD * DH], bf, kind="ExternalInput")
    wk_d = nc.dram_tensor("wk", [128, NKD * DH], bf, kind="ExternalInput")
    wv_d = nc.dram_tensor("wv", [128, NKD * DH], bf, kind="ExternalInput")
    wo_d = nc.dram_tensor("wo", [128, NKO * D], bf, kind="ExternalInput")
    bq_d = nc.dram_tensor("bq", [1, DH], bf, kind="ExternalInput")
    bk_d = nc.dram_tensor("bk", [1, DH], bf, kind="ExternalInput")
    bv_d = nc.dram_tensor("bv", [1, DH], bf, kind="ExternalInput")
    out_d = nc.dram_tensor("out", [NHP, D, S], f32, kind="ExternalOutput")

    with tile.TileContext(nc) as tc:
        _build_body(nc, tc, S, NSC, NQT, NKD, NKO, NMO, NHP,
                    xq_d, xk_d, xv_d, wq_d, wk_d, wv_d, wo_d,
                    bq_d, bk_d, bv_d, out_d, has_bias=has_bias)
    nc.compile()
    return nc


def _build_body(nc, tc, S, NSC, NQT, NKD, NKO, NMO, NHP,
                xq_d, xk_d, xv_d, wq_d, wk_d, wv_d, wo_d,
                bq_d, bk_d, bv_d, out_d, has_bias=True):
    from contextlib import ExitStack
    ctx = ExitStack()
    with ctx:
        consts = ctx.enter_context(tc.tile_pool(name="consts", bufs=1))
        wpool = ctx.enter_context(tc.tile_pool(name="wpool", bufs=1))
        xqk_pool = ctx.enter_context(tc.tile_pool(name="xqk", bufs=1))
        xv_pool = ctx.enter_context(tc.tile_pool(name="xvp", bufs=NKD))
        p_pool = ctx.enter_context(tc.tile_pool(name="pp", bufs=3 if not has_bias else 2))
        vpool = ctx.enter_context(tc.tile_pool(name="vpool", bufs=1))
        qk_pool = ctx.enter_context(tc.tile_pool(name="qkT", bufs=2))
        ctx_pool = ctx.enter_context(tc.tile_pool(name="ctxp", bufs=1))
        norm_pool = ctx.enter_context(tc.tile_pool(name="normp", bufs=3))
        out_pool = ctx.enter_context(tc.tile_pool(name="outp", bufs=2))
        dram_pool = ctx.enter_context(tc.tile_pool(name="dramp", bufs=2, space="DRAM"))
        psum_s = ctx.enter_context(tc.tile_pool(name="psum_s", bufs=2, space="PSUM"))
        psum_c = ctx.enter_context(tc.tile_pool(name="psum_c", bufs=2, space="PSUM"))
        psum_ctx = ctx.enter_context(tc.tile_pool(name="psum_ctx", bufs=2, space="PSUM"))

        # ---- constants / weights -------------------------------------------
        ones_row = consts.tile([1, 512], bf)
        nc.vector.memset(ones_row[:], 1.0)

        wq_sb = wpool.tile([128, NKD, DH], bf, tag="wq")
        wk_sb = wpool.tile([128, NKD, DH], bf, tag="wk")
        wv_sb = wpool.tile([128, NKD, DH], bf, tag="wv")
        bq_sb = bk_sb = bv_sb = None
        if has_bias:
            bq_sb = consts.tile([1, DH], bf, tag="bq")
            bk_sb = consts.tile([1, DH], bf, tag="bk")
            bv_sb = consts.tile([1, DH], bf, tag="bv")

        # load order matters: qk-projection inputs first so PE starts early,
        # then V inputs (consumed shortly after), wo last.
        nc.sync.dma_start(out=wk_sb[:], in_=wk_d[:].rearrange("p (c n) -> p c n", c=NKD))
        nc.sync.dma_start(out=wq_sb[:], in_=wq_d[:].rearrange("p (c n) -> p c n", c=NKD))
        if has_bias:
            nc.sync.dma_start(out=bq_sb[:], in_=bq_d[:])
            nc.sync.dma_start(out=bk_sb[:], in_=bk_d[:])
            nc.sync.dma_start(out=bv_sb[:], in_=bv_d[:])
        xq_tiles, xk_tiles = [], []
        for k in range(NKD):
            tk = xqk_pool.tile([128, S], bf, name=f"xk{k}", tag=f"xk{k}")
            nc.sync.dma_start(out=tk[:], in_=xk_d[128 * k:128 * (k + 1), :])
            xk_tiles.append(tk)
        for k in range(NKD):
            tq = xqk_pool.tile([128, S], bf, name=f"xq{k}", tag=f"xq{k}")
            nc.sync.dma_start(out=tq[:], in_=xq_d[128 * k:128 * (k + 1), :])
            xq_tiles.append(tq)
        nc.sync.dma_start(out=wv_sb[:], in_=wv_d[:].rearrange("p (c n) -> p c n", c=NKD))
        xv_tiles = []
        for k in range(NKD):
            t = xv_pool.tile([128, S], bf, name=f"xv{k}", tag="xvt")
            nc.sync.dma_start(out=t[:], in_=xv_d[128 * k:128 * (k + 1), :])
            xv_tiles.append(t)

        v_sb = [None] * NSC

        def emit_v_chunk(s):
            ps = psum_c.tile([128, 512], f32, name=f"vps{s}", tag="ps512")
            for k in range(NKD):
                nc.tensor.matmul(
                    ps[:], xv_tiles[k][:, 128 * s:128 * (s + 1)], wv_sb[:, k, :],
                    start=(k == 0), stop=(not has_bias and k == NKD - 1))
            if has_bias:
                nc.tensor.matmul(ps[:], ones_row[0:1, 0:128], bv_sb[:],
                                 start=False, stop=True)
            vt = vpool.tile([128, H_CORE, DK + 1], bf, name=f"v{s}", tag=f"v{s}")
            nc.vector.tensor_copy(
                out=vt[:, :, 0:DK],
                in_=ps[:].rearrange("p (h d) -> p h d", h=H_CORE))
            nc.vector.memset(vt[:, :, DK:DK + 1], 1.0)
            v_sb[s] = vt

        def emit_proj_ntile(w_sb, b_sb, x_tiles, dst, hp, n):
            ps = psum_c.tile([128, 512], f32, name=f"pj{hp}{n}", tag="ps512")
            for k in range(NKD):
                nc.tensor.matmul(
                    ps[:],
                    w_sb[:, k, 128 * hp:128 * (hp + 1)],
                    x_tiles[k][:, 512 * n:512 * (n + 1)],
                    start=(k == 0), stop=(not has_bias and k == NKD - 1))
            if has_bias:
                nc.tensor.matmul(
                    ps[:], b_sb[0:1, 128 * hp:128 * (hp + 1)],
                    ones_row[0:1, 0:512],
                    start=False, stop=True)
            nc.vector.tensor_copy(out=dst[:, 512 * n:512 * (n + 1)], in_=ps[:])

        ctx_sb = [ctx_pool.tile([128, S], bf, name=f"ctx{hp}", tag=f"ctx{hp}")
                  for hp in range(NHP)]

        def emit_outproj_pass(hp):
            # this head-pair's contribution to out^T -> its own DRAM slice
            kc = hp
            t = wpool.tile([128, D], bf, name=f"wo{kc}", tag=f"wo{kc}", bufs=1)
            nc.sync.dma_start(out=t[:], in_=wo_d[:, D * kc:D * (kc + 1)])
            pools = [(psum_c, "ps512"), (psum_s, "st"), (psum_ctx, "cps")]
            for mo in range(NMO):
                for nt in range(NQT):
                    i_t = mo * NQT + nt
                    pool, ptag = pools[i_t % 3]
                    ps = pool.tile([128, 512], f32, name=f"ops{hp}{mo}{nt}", tag=ptag)
                    nc.tensor.matmul(
                        ps[:],
                        t[:, 128 * mo:128 * (mo + 1)],
                        ctx_sb[kc][:, 512 * nt:512 * (nt + 1)],
                        start=True, stop=True)
                    ot = out_pool.tile([128, 512], f32, name=f"o{mo}{nt}", tag="ot")
                    if i_t % 2 == 0:
                        nc.vector.tensor_copy(out=ot[:], in_=ps[:])
                    else:
                        nc.scalar.copy(out=ot[:], in_=ps[:])
                    dst = out_d[hp, 128 * mo:128 * (mo + 1),
                                512 * nt:512 * (nt + 1)]
                    nc.sync.dma_start(out=dst, in_=ot[:])

        # ---- per head pair: Q^T/K^T projection then attention --------------
        for hp in range(NHP):
            qT = qk_pool.tile([128, S], bf, name=f"qT{hp}", tag="qT")
            kT = qk_pool.tile([128, S], bf, name=f"kT{hp}", tag="kT")
            if hp > 0:
                for n in range(NQT):
                    emit_proj_ntile(wk_sb, bk_sb, xk_tiles, kT, hp, n)
                for n in range(NQT):
                    emit_proj_ntile(wq_sb, bq_sb, xq_tiles, qT, hp, n)
            else:
                emit_proj_ntile(wk_sb, bk_sb, xk_tiles, kT, 0, 0)

            # attention for heads (2hp, 2hp+1); local row offsets 0 / 64
            # sums rows staged at partitions {0,32,64,96} of two tiles
            # (the only legal AP start partitions), then batch-reciprocaled.
            ctxu_tiles = []
            stage = [norm_pool.tile([128, 512], f32, name=f"stage{hp}_{i}",
                                    tag="stage", bufs=2) for i in range(2)]
            for stg in stage:
                nc.vector.memset(stg[:], 1.0)
            for qt in range(NQT):
                if hp == 0:
                    emit_proj_ntile(wq_sb, bq_sb, xq_tiles, qT, hp, qt)
                cps0 = psum_ctx.tile([128, 512], f32, name=f"c0_{hp}_{qt}", tag="cps")
                cps1 = psum_ctx.tile([128, 512], f32, name=f"c1_{hp}_{qt}", tag="cps")
                for kc in range(NSC):
                    if hp == 0 and qt == 0:
                        if kc % 4 == 3 and kc < NSC - 1:
                            emit_proj_ntile(wk_sb, bk_sb, xk_tiles, kT, 0,
                                            (kc + 1) // 4)
                        emit_v_chunk(kc)
                    st = psum_s.tile([128, 1024], f32, name=f"st{hp}{qt}{kc}", tag="st")
                    for j in range(2):  # head within pair
                        r0 = 64 * j
                        nc.tensor.matmul(
                            st[:, 512 * j:512 * (j + 1)],
                            kT[r0:r0 + 64, 128 * kc:128 * (kc + 1)],
                            qT[r0:r0 + 64, 512 * qt:512 * (qt + 1)],
                            start=True, stop=True,
                            tile_position=(r0, 0))
                    pt = p_pool.tile([128, 1024], bf, name=f"p{hp}{qt}{kc}",
                                     tag="pp")
                    nc.scalar.activation(out=pt[:], in_=st[:],
                                         func=mybir.ActivationFunctionType.Exp,
                                         scale=0.125)
                    for j, cps in enumerate((cps0, cps1)):
                        nc.tensor.matmul(
                            cps[0:DK + 1, :],
                            v_sb[kc][:, 2 * hp + j, :],
                            pt[:, 512 * j:512 * (j + 1)],
                            start=(kc == 0), stop=(kc == NSC - 1))
                # evict unnormalized ctx + sums to SBUF, freeing PSUM fast
                for j, cps in enumerate((cps0, cps1)):
                    r = 2 * qt + j
                    nc.vector.tensor_copy(
                        out=stage[r // 4][32 * (r % 4):32 * (r % 4) + 1, :],
                        in_=cps[DK:DK + 1, :])
                    cu = norm_pool.tile([DK, 512], bf,
                                        name=f"cu{hp}{qt}{j}", tag="cu", bufs=8)
                    nc.vector.tensor_copy(out=cu[:], in_=cps[0:DK, :])
                    ctxu_tiles.append(cu)
            # batched reciprocal of all 8 (head, qtile) sums rows
            recips = [norm_pool.tile([128, 512], f32, name=f"rc{hp}_{i}",
                                     tag="recips", bufs=1) for i in range(2)]
            for i in range(2):
                nc.vector.reciprocal(out=recips[i][:], in_=stage[i][:])
            # bounce through DRAM: HW partition-broadcast only reads p0, but a
            # DRAM-source DMA with partition step 0 broadcasts any row.
            rdram = dram_pool.tile([2 * NQT, 512], f32, name=f"rd{hp}", tag="rd")
            for r in range(2 * NQT):
                nc.sync.dma_start(
                    out=rdram[r:r + 1, :],
                    in_=recips[r // 4][32 * (r % 4):32 * (r % 4) + 1, :])
            for qt in range(NQT):
                for j in range(2):
                    r = 2 * qt + j
                    bcast = norm_pool.tile([DK, 512], f32, name=f"b{hp}{qt}{j}",
                                           tag="bcast", bufs=2)
                    nc.sync.dma_start(out=bcast[:],
                                      in_=rdram[r:r + 1, :].to_broadcast([DK, 512]))
                    nc.vector.tensor_mul(
                        out=ctx_sb[hp][64 * j:64 * (j + 1),
                                       512 * qt:512 * (qt + 1)],
                        in0=ctxu_tiles[r][:], in1=bcast[:])
            emit_outproj_pass(hp)



# ----------------------------------------------------------------------------
# host side
# ----------------------------------------------------------------------------

def _tile_w(wT, nchunks):
    """[K, M] -> [128, nchunks*M] partition-major bf16."""
    K, M = wT.shape
    assert K == nchunks * 128
    return np.ascontiguousarray(
        wT.reshape(nchunks, 128, M).transpose(1, 0, 2).reshape(128, nchunks * M)
    ).astype(BF16)


def make_in_maps(query, key, value, Wq, bq, Wk, bk, Wv, bv, Wo, S=2048):
    in_maps = []
    for c in range(N_CORES):
        b, g = divmod(c, 2)
        hd = slice(DH * g, DH * (g + 1))
        m = {
            "xq": np.ascontiguousarray(query[:, b, :].T).astype(BF16),
            "xk": np.ascontiguousarray(key[:, b, :].T).astype(BF16),
            "xv": np.ascontiguousarray(value[:, b, :].T).astype(BF16),
            "wq": _tile_w(np.ascontiguousarray(Wq[hd, :].T), D // 128),
            "wk": _tile_w(np.ascontiguousarray(Wk[hd, :].T), D // 128),
            "wv": _tile_w(np.ascontiguousarray(Wv[hd, :].T), D // 128),
            "wo": _tile_w(np.ascontiguousarray(Wo[:, hd].T), DH // 128),
            "bq": bq[hd].reshape(1, DH).astype(BF16),
            "bk": bk[hd].reshape(1, DH).astype(BF16),
            "bv": bv[hd].reshape(1, DH).astype(BF16),
        }
        in_maps.append(m)
    return in_maps


def combine_outputs(results, bo, S=2048):
    out = np.empty((S, 4, D), np.float32)
    for b in range(4):
        pT = (results[2 * b]["out"].sum(axis=0)
              + results[2 * b + 1]["out"].sum(axis=0))  # [D, S]
        out[:, b, :] = pT.T
    out += bo.astype(np.float32)[None, None, :]
    return out


_NC_CACHE = {}


def get_program(S=2048, has_bias=True):
    key = (S, has_bias)
    if key not in _NC_CACHE:
        _NC_CACHE[key] = build_program(S, has_bias=has_bias)
    return _NC_CACHE[key]


def kernel(query, key, value, Wq, bq, Wk, bk, Wv, bv, Wo, bo, **_):
    query = np.asarray(query, np.float32)
    key = np.asarray(key, np.float32)
    value = np.asarray(value, np.float32)
    S = query.shape[0]
    has_bias = any(float(np.abs(np.asarray(b)).max()) != 0.0
                   for b in (bq, bk, bv))
    nc = get_program(S, has_bias=has_bias)
    in_maps = make_in_maps(query, key, value,
                           np.asarray(Wq), np.asarray(bq),
                           np.asarray(Wk), np.asarray(bk),
                           np.asarray(Wv), np.asarray(bv),
                           np.asarray(Wo), S=S)
    res = run_bass_kernel_spmd(nc, in_maps, core_ids=list(range(N_CORES)))
    return combine_outputs(res.results, np.asarray(bo), S=S)


# revision 28
# speedup vs baseline: 1.5513x; 1.5513x over previous
"""Trainium2 Bass kernel for nn_MultiHeadAttention (S=2048, B=4, D=1024, H=16).

Sharding: 8 cores = 4 batches x 2 head-groups (8 heads each).
Each core computes, for its (batch b, head-group g):
  Q^T/K^T = Wslice @ x^T (dim-major), V = x @ Wslice^T (seq-major, + ones col)
  S^T = K_h @ Q_h^T  (keys on partitions), P^T = exp(S^T/8)  (ACT, bf16)
  ctx^T+sums = [V_h|1]^T @ P^T  (PSUM accum over key chunks)
  ctx = ctx_unnorm * (1/sums)   (DVE recip + DMA broadcast)
  out^T_partial = Wo_slice^T.T @ ctx^T
Host: out[:, b, :] = (partial[2b] + partial[2b+1]).T + bo
"""

import sys

for _p in ("/opt/trn_rl_repo", "/opt/pypackages"):
    if _p not in sys.path:
        sys.path.append(_p)

import numpy as np
import ml_dtypes

import concourse.bacc as bacc
import concourse.tile as tile
from concourse import mybir
from concourse.bass_utils import run_bass_kernel_spmd

BF16 = ml_dtypes.bfloat16
FP32 = np.float32

D = 1024          # model dim
H_CORE = 8        # heads per core
DK = 64           # head dim
DH = H_CORE * DK  # 512 per-core head dims
N_CORES = 8

bf = mybir.dt.bfloat16
f32 = mybir.dt.float32


def build_program(S=2048, has_bias=True):
    """Build the per-core SPMD Bass program. Returns (nc, names)."""
    assert S % 512 == 0
    NSC = S // 128    # seq/key chunks of 128
    NQT = S // 512    # q tiles of 512
    NKD = D // 128    # 8 model-dim contraction chunks
    NKO = DH // 128   # 4 ctx contraction chunks
    NMO = D // 128    # 8 out-dim chunks
    NHP = H_CORE // 2 # 4 head pairs

    nc = bacc.Bacc(None, target_bir_lowering=False)

    xq_d = nc.dram_tensor("xq", [D, S], bf, kind="ExternalInput")
    xk_d = nc.dram_tensor("xk", [D, S], bf, kind="ExternalInput")
    xv_d = nc.dram_tensor("xv", [D, S], bf, kind="ExternalInput")
    # weights pre-tiled on host to [128, nchunks*cols] partition-major
    wq_d = nc.dram_tensor("wq", [128, NKD * DH], bf, kind="ExternalInput")
    wk_d = nc.dram_tensor("wk", [128, NKD * DH], bf, kind="ExternalInput")
    wv_d = nc.dram_tensor("wv", [128, NKD * DH], bf, kind="ExternalInput")
    wo_d = nc.dram_tensor("wo", [128, NKO * D], bf, kind="ExternalInput")
    bq_d = nc.dram_tensor("bq", [1, DH], bf, kind="ExternalInput")
    bk_d = nc.dram_tensor("bk", [1, DH], bf, kind="ExternalInput")
    bv_d = nc.dram_tensor("bv", [1, DH], bf, kind="ExternalInput")
    out_d = nc.dram_tensor("out", [D, S], f32, kind="ExternalOutput")

    with tile.TileContext(nc) as tc:
        _build_body(nc, tc, S, NSC, NQT, NKD, NKO, NMO, NHP,
                    xq_d, xk_d, xv_d, wq_d, wk_d, wv_d, wo_d,
                    bq_d, bk_d, bv_d, out_d, has_bias=has_bias)
    nc.compile()
    return nc


def _build_body(nc, tc, S, NSC, NQT, NKD, NKO, NMO, NHP,
                xq_d, xk_d, xv_d, wq_d, wk_d, wv_d, wo_d,
                bq_d, bk_d, bv_d, out_d, has_bias=True):
    from contextlib import ExitStack
    ctx = ExitStack()
    with ctx:
        consts = ctx.enter_context(tc.tile_pool(name="consts", bufs=1))
        wpool = ctx.enter_context(tc.tile_pool(name="wpool", bufs=1))
        xqk_pool = ctx.enter_context(tc.tile_pool(name="xqk", bufs=1))
        xv_pool = ctx.enter_context(tc.tile_pool(name="xvp", bufs=NKD))
        p_pool = ctx.enter_context(tc.tile_pool(name="pp", bufs=3))
        vpool = ctx.enter_context(tc.tile_pool(name="vpool", bufs=1))
        qk_pool = ctx.enter_context(tc.tile_pool(name="qkT", bufs=2))
        ctx_pool = ctx.enter_context(tc.tile_pool(name="ctxp", bufs=1))
        norm_pool = ctx.enter_context(tc.tile_pool(name="normp", bufs=3))
        out_pool = ctx.enter_context(tc.tile_pool(name="outp", bufs=2))
        dram_pool = ctx.enter_context(tc.tile_pool(name="dramp", bufs=2, space="DRAM"))
        psum_s = ctx.enter_context(tc.tile_pool(name="psum_s", bufs=2, space="PSUM"))
        psum_c = ctx.enter_context(tc.tile_pool(name="psum_c", bufs=2, space="PSUM"))
        psum_ctx = ctx.enter_context(tc.tile_pool(name="psum_ctx", bufs=2, space="PSUM"))

        # ---- constants / weights -------------------------------------------
        ones_row = consts.tile([1, 512], bf)
        nc.vector.memset(ones_row[:], 1.0)

        wq_sb = wpool.tile([128, NKD, DH], bf, tag="wq")
        wk_sb = wpool.tile([128, NKD, DH], bf, tag="wk")
        wv_sb = wpool.tile([128, NKD, DH], bf, tag="wv")
        bq_sb = bk_sb = bv_sb = None
        if has_bias:
            bq_sb = consts.tile([1, DH], bf, tag="bq")
            bk_sb = consts.tile([1, DH], bf, tag="bk")
            bv_sb = consts.tile([1, DH], bf, tag="bv")

        # load order: K-proj inputs first so PE starts earliest, then Q, V; wo last
        nc.sync.dma_start(out=wk_sb[:], in_=wk_d[:].rearrange("p (c n) -> p c n", c=NKD))
        xq_tiles, xk_tiles = [], []
        for k in range(NKD):
            tk = xqk_pool.tile([128, S], bf, name=f"xk{k}", tag=f"xk{k}")
            nc.sync.dma_start(out=tk[:], in_=xk_d[128 * k:128 * (k + 1), :])
            xk_tiles.append(tk)
        nc.sync.dma_start(out=wq_sb[:], in_=wq_d[:].rearrange("p (c n) -> p c n", c=NKD))
        if has_bias:
            nc.sync.dma_start(out=bq_sb[:], in_=bq_d[:])
            nc.sync.dma_start(out=bk_sb[:], in_=bk_d[:])
            nc.sync.dma_start(out=bv_sb[:], in_=bv_d[:])
        for k in range(NKD):
            tq = xqk_pool.tile([128, S], bf, name=f"xq{k}", tag=f"xq{k}")
            nc.sync.dma_start(out=tq[:], in_=xq_d[128 * k:128 * (k + 1), :])
            xq_tiles.append(tq)
        nc.sync.dma_start(out=wv_sb[:], in_=wv_d[:].rearrange("p (c n) -> p c n", c=NKD))
        xv_tiles = []
        for k in range(NKD):
            t = xv_pool.tile([128, S], bf, name=f"xv{k}", tag="xvt")
            nc.sync.dma_start(out=t[:], in_=xv_d[128 * k:128 * (k + 1), :])
            xv_tiles.append(t)

        v_sb = [None] * NSC

        def emit_v_chunk(s):
            ps = psum_c.tile([128, 512], f32, name=f"vps{s}", tag="ps512")
            for k in range(NKD):
                nc.tensor.matmul(
                    ps[:], xv_tiles[k][:, 128 * s:128 * (s + 1)], wv_sb[:, k, :],
                    start=(k == 0), stop=(not has_bias and k == NKD - 1))
            if has_bias:
                nc.tensor.matmul(ps[:], ones_row[0:1, 0:128], bv_sb[:],
                                 start=False, stop=True)
            vt = vpool.tile([128, H_CORE, DK + 1], bf, name=f"v{s}", tag=f"v{s}")
            nc.vector.tensor_copy(
                out=vt[:, :, 0:DK],
                in_=ps[:].rearrange("p (h d) -> p h d", h=H_CORE))
            nc.vector.memset(vt[:, :, DK:DK + 1], 1.0)
            v_sb[s] = vt

        def emit_proj_ntile(args):
            w_sb, b_sb, x_tiles, dst, hp, n = args
            ps = psum_c.tile([128, 512], f32, name=f"pj{hp}{n}", tag="ps512")
            for k in range(NKD):
                nc.tensor.matmul(
                    ps[:],
                    w_sb[:, k, 128 * hp:128 * (hp + 1)],
                    x_tiles[k][:, 512 * n:512 * (n + 1)],
                    start=(k == 0), stop=(not has_bias and k == NKD - 1))
            if has_bias:
                nc.tensor.matmul(
                    ps[:], b_sb[0:1, 128 * hp:128 * (hp + 1)],
                    ones_row[0:1, 0:512],
                    start=False, stop=True)
            nc.vector.tensor_copy(out=dst[:, 512 * n:512 * (n + 1)], in_=ps[:])

        ctx_sb = [ctx_pool.tile([128, S], bf, name=f"ctx{hp}", tag=f"ctx{hp}")
                  for hp in range(NHP)]

        # qT/kT tiles allocated one hp ahead (projections are pipelined into
        # the previous head pair's attention loop)
        qk_tiles = {}

        def get_qk(hp):
            if hp not in qk_tiles:
                qT = qk_pool.tile([128, S], bf, name=f"qT{hp}", tag="qT")
                kT = qk_pool.tile([128, S], bf, name=f"kT{hp}", tag="kT")
                qk_tiles[hp] = (qT, kT)
            return qk_tiles[hp]

        # ---- per head pair: attention (with pipelined proj of hp+1) --------
        for hp in range(NHP):
            qT, kT = get_qk(hp)
            if hp == 0:
                emit_proj_ntile((wk_sb, bk_sb, xk_tiles, kT, 0, 0))
            # work queue of next-hp projection tiles to sprinkle into qt2/qt3
            pending = []
            if hp + 1 < NHP:
                nqT, nkT = get_qk(hp + 1)
                for n in range(NQT):
                    pending.append((wk_sb, bk_sb, xk_tiles, nkT, hp + 1, n))
                for n in range(NQT):
                    pending.append((wq_sb, bq_sb, xq_tiles, nqT, hp + 1, n))

            cu_store = {}
            for qt in range(NQT):
                if hp == 0:
                    emit_proj_ntile((wq_sb, bq_sb, xq_tiles, qT, 0, qt))
                cps0 = psum_ctx.tile([128, 512], f32, name=f"c0_{hp}_{qt}", tag="cps")
                cps1 = psum_ctx.tile([128, 512], f32, name=f"c1_{hp}_{qt}", tag="cps")
                for kc in range(NSC):
                    if hp == 0 and qt == 0:
                        if kc % 4 == 3 and kc < NSC - 1:
                            emit_proj_ntile((wk_sb, bk_sb, xk_tiles, kT, 0,
                                             (kc + 1) // 4))
                        emit_v_chunk(kc)
                    elif pending and qt >= 2 and kc % 4 == 0:
                        emit_proj_ntile(pending.pop(0))
                    st = psum_s.tile([128, 1024], f32, name=f"st{hp}{qt}{kc}", tag="st")
                    for j in range(2):  # head within pair
                        r0 = 64 * j
                        nc.tensor.matmul(
                            st[:, 512 * j:512 * (j + 1)],
                            kT[r0:r0 + 64, 128 * kc:128 * (kc + 1)],
                            qT[r0:r0 + 64, 512 * qt:512 * (qt + 1)],
                            start=True, stop=True,
                            tile_position=(r0, 0))
                    pt = p_pool.tile([128, 1024], bf, name=f"p{hp}{qt}{kc}",
                                     tag="pp")
                    nc.scalar.activation(out=pt[:], in_=st[:],
                                         func=mybir.ActivationFunctionType.Exp,
                                         scale=0.125)
                    for j, cps in enumerate((cps0, cps1)):
                        nc.tensor.matmul(
                            cps[0:DK + 1, :],
                            v_sb[kc][:, 2 * hp + j, :],
                            pt[:, 512 * j:512 * (j + 1)],
                            start=(kc == 0), stop=(kc == NSC - 1))
                # evict unnormalized ctx + sums; stage sums at {0,32,64,96}
                stg_i = qt // 2
                if qt % 2 == 0:
                    cu_store[f"stage{stg_i}"] = norm_pool.tile(
                        [128, 512], f32, name=f"stage{hp}_{stg_i}",
                        tag="stage", bufs=2)
                    nc.vector.memset(cu_store[f"stage{stg_i}"][:], 1.0)
                stage = cu_store[f"stage{stg_i}"]
                for j, cps in enumerate((cps0, cps1)):
                    r = 2 * (qt % 2) + j
                    nc.vector.tensor_copy(
                        out=stage[32 * r:32 * r + 1, :],
                        in_=cps[DK:DK + 1, :])
                    cu = norm_pool.tile([DK, 512], bf,
                                        name=f"cu{hp}{qt}{j}", tag="cu", bufs=5)
                    nc.vector.tensor_copy(out=cu[:], in_=cps[0:DK, :])
                    cu_store[(qt, j)] = cu
                if qt % 2 == 1 or qt == NQT - 1:
                    # normalize this q-tile pair's (qt, j) rows
                    recips = norm_pool.tile([128, 512], f32,
                                            name=f"rc{hp}_{stg_i}",
                                            tag="recips", bufs=2)
                    nc.vector.reciprocal(out=recips[:], in_=stage[:])
                    rdram = dram_pool.tile([4, 512], f32,
                                           name=f"rd{hp}{stg_i}", tag="rd")
                    for r in range(4):
                        nc.sync.dma_start(
                            out=rdram[r:r + 1, :],
                            in_=recips[32 * r:32 * r + 1, :])
                    for qtt in ([qt] if qt % 2 == 0 else [qt - 1, qt]):
                        for j in range(2):
                            r = 2 * (qtt % 2) + j
                            bcast = norm_pool.tile(
                                [DK, 512], f32, name=f"b{hp}{qtt}{j}",
                                tag="bcast", bufs=2)
                            nc.sync.dma_start(
                                out=bcast[:],
                                in_=rdram[r:r + 1, :].to_broadcast([DK, 512]))
                            nc.vector.tensor_mul(
                                out=ctx_sb[hp][64 * j:64 * (j + 1),
                                               512 * qtt:512 * (qtt + 1)],
                                in0=cu_store[(qtt, j)][:], in1=bcast[:])
            # small-S configs: drain any proj work not absorbed by qt>=2 slots
            for args in pending:
                emit_proj_ntile(args)

        # ---- output projection (single pass, psum-accumulated) -------------
        wo_t = {}
        for kc in range(NKO):
            t = wpool.tile([128, D], bf, name=f"wo{kc}", tag=f"wo{kc}", bufs=1)
            nc.sync.dma_start(out=t[:], in_=wo_d[:, D * kc:D * (kc + 1)])
            wo_t[kc] = t
        pools = [(psum_c, "ps512"), (psum_s, "st"), (psum_ctx, "cps")]
        for mo in range(NMO):
            for nt in range(NQT):
                i_t = mo * NQT + nt
                pool, ptag = pools[i_t % 3]
                ps = pool.tile([128, 512], f32, name=f"ops{mo}{nt}", tag=ptag)
                for kc in range(NKO):
                    nc.tensor.matmul(
                        ps[:],
                        wo_t[kc][:, 128 * mo:128 * (mo + 1)],
                        ctx_sb[kc][:, 512 * nt:512 * (nt + 1)],
                        start=(kc == 0), stop=(kc == NKO - 1))
                ot = out_pool.tile([128, 512], f32, name=f"o{mo}{nt}", tag="ot")
                if i_t % 2 == 0:
                    nc.vector.tensor_copy(out=ot[:], in_=ps[:])
                else:
                    nc.scalar.copy(out=ot[:], in_=ps[:])
                dst = out_d[128 * mo:128 * (mo + 1), 512 * nt:512 * (nt + 1)]
                nc.sync.dma_start(out=dst, in_=ot[:])


# ----------------------------------------------------------------------------
# host side
# ----------------------------------------------------------------------------

def _tile_w(wT, nchunks):
    """[K, M] -> [128, nchunks*M] partition-major bf16."""
    K, M = wT.shape
    assert K == nchunks * 128
    return np.ascontiguousarray(
        wT.reshape(nchunks, 128, M).transpose(1, 0, 2).reshape(128, nchunks * M)
    ).astype(BF16)


def make_in_maps(query, key, value, Wq, bq, Wk, bk, Wv, bv, Wo, S=2048):
    in_maps = []
    for c in range(N_CORES):
        b, g = divmod(c, 2)
        hd = slice(DH * g, DH * (g + 1))
        m = {
            "xq": np.ascontiguousarray(query[:, b, :].T).astype(BF16),
            "xk": np.ascontiguousarray(key[:, b, :].T).astype(BF16),
            "xv": np.ascontiguousarray(value[:, b, :].T).astype(BF16),
            "wq": _tile_w(np.ascontiguousarray(Wq[hd, :].T), D // 128),
            "wk": _tile_w(np.ascontiguousarray(Wk[hd, :].T), D // 128),
            "wv": _tile_w(np.ascontiguousarray(Wv[hd, :].T), D // 128),
            "wo": _tile_w(np.ascontiguousarray(Wo[:, hd].T), DH // 128),
            "bq": bq[hd].reshape(1, DH).astype(BF16),
            "bk": bk[hd].reshape(1, DH).astype(BF16),
            "bv": bv[hd].reshape(1, DH).astype(BF16),
        }
        in_maps.append(m)
    return in_maps


def combine_outputs(results, bo, S=2048):
    out = np.empty((S, 4, D), np.float32)
    for b in range(4):
        pT = results[2 * b]["out"] + results[2 * b + 1]["out"]  # [D, S]
        out[:, b, :] = pT.T
    out += bo.astype(np.float32)[None, None, :]
    return out


_NC_CACHE = {}


def get_program(S=2048, has_bias=True):
    key = (S, has_bias)
    if key not in _NC_CACHE:
        _NC_CACHE[key] = build_program(S, has_bias=has_bias)
    return _NC_CACHE[key]


def kernel(query, key, value, Wq, bq, Wk, bk, Wv, bv, Wo, bo, **_):
    query = np.asarray(query, np.float32)
    key = np.asarray(key, np.float32)
    value = np.asarray(value, np.float32)
    S = query.shape[0]
    has_bias = any(float(np.abs(np.asarray(b)).max()) != 0.0
                   for b in (bq, bk, bv))
    nc = get_program(S, has_bias=has_bias)
    in_maps = make_in_maps(query, key, value,
                           np.asarray(Wq), np.asarray(bq),
                           np.asarray(Wk), np.asarray(bk),
                           np.asarray(Wv), np.asarray(bv),
                           np.asarray(Wo), S=S)
    res = run_bass_kernel_spmd(nc, in_maps, core_ids=list(range(N_CORES)))
    return combine_outputs(res.results, np.asarray(bo), S=S)


# revision 29
# speedup vs baseline: 1.6189x; 1.0435x over previous
"""Trainium2 Bass kernel for nn_MultiHeadAttention (S=2048, B=4, D=1024, H=16).

Sharding: 8 cores = 4 batches x 2 head-groups (8 heads each).
Each core computes, for its (batch b, head-group g):
  Q^T/K^T = Wslice @ x^T (dim-major), V = x @ Wslice^T (seq-major, + ones col)
  S^T = K_h @ Q_h^T  (keys on partitions), P^T = exp(S^T/8)  (ACT, bf16)
  ctx^T+sums = [V_h|1]^T @ P^T  (PSUM accum over key chunks)
  ctx = ctx_unnorm * (1/sums)   (DVE recip + DMA broadcast)
  out^T_partial = Wo_slice^T.T @ ctx^T
Host: out[:, b, :] = (partial[2b] + partial[2b+1]).T + bo
"""

import sys

for _p in ("/opt/trn_rl_repo", "/opt/pypackages"):
    if _p not in sys.path:
        sys.path.append(_p)

import numpy as np
import ml_dtypes

import concourse.bacc as bacc
import concourse.tile as tile
from concourse import mybir
from concourse.bass_utils import run_bass_kernel_spmd

BF16 = ml_dtypes.bfloat16
FP32 = np.float32

D = 1024          # model dim
H_CORE = 8        # heads per core
DK = 64           # head dim
DH = H_CORE * DK  # 512 per-core head dims
N_CORES = 8

bf = mybir.dt.bfloat16
f32 = mybir.dt.float32


def build_program(S=2048, has_bias=True):
    """Build the per-core SPMD Bass program. Returns (nc, names)."""
    assert S % 512 == 0
    NSC = S // 128    # seq/key chunks of 128
    NQT = S // 512    # q tiles of 512
    NKD = D // 128    # 8 model-dim contraction chunks
    NKO = DH // 128   # 4 ctx contraction chunks
    NMO = D // 128    # 8 out-dim chunks
    NHP = H_CORE // 2 # 4 head pairs

    nc = bacc.Bacc(None, target_bir_lowering=False)

    xq_d = nc.dram_tensor("xq", [D, S], bf, kind="ExternalInput")
    xk_d = nc.dram_tensor("xk", [D, S], bf, kind="ExternalInput")
    xv_d = nc.dram_tensor("xv", [D, S], bf, kind="ExternalInput")
    # weights pre-tiled on host to [128, nchunks*cols] partition-major
    wq_d = nc.dram_tensor("wq", [128, NKD * DH], bf, kind="ExternalInput")
    wk_d = nc.dram_tensor("wk", [128, NKD * DH], bf, kind="ExternalInput")
    wv_d = nc.dram_tensor("wv", [128, NKD * DH], bf, kind="ExternalInput")
    wo_d = nc.dram_tensor("wo", [128, NKO * D], bf, kind="ExternalInput")
    bq_d = nc.dram_tensor("bq", [1, DH], bf, kind="ExternalInput")
    bk_d = nc.dram_tensor("bk", [1, DH], bf, kind="ExternalInput")
    bv_d = nc.dram_tensor("bv", [1, DH], bf, kind="ExternalInput")
    out_d = nc.dram_tensor("out", [D, S], f32, kind="ExternalOutput")

    with tile.TileContext(nc) as tc:
        _build_body(nc, tc, S, NSC, NQT, NKD, NKO, NMO, NHP,
                    xq_d, xk_d, xv_d, wq_d, wk_d, wv_d, wo_d,
                    bq_d, bk_d, bv_d, out_d, has_bias=has_bias)
    nc.compile()
    return nc


def _build_body(nc, tc, S, NSC, NQT, NKD, NKO, NMO, NHP,
                xq_d, xk_d, xv_d, wq_d, wk_d, wv_d, wo_d,
                bq_d, bk_d, bv_d, out_d, has_bias=True):
    from contextlib import ExitStack
    ctx = ExitStack()
    with ctx:
        consts = ctx.enter_context(tc.tile_pool(name="consts", bufs=1))
        wpool = ctx.enter_context(tc.tile_pool(name="wpool", bufs=1))
        xqk_pool = ctx.enter_context(tc.tile_pool(name="xqk", bufs=1))
        xv_pool = ctx.enter_context(tc.tile_pool(name="xvp", bufs=NKD))
        p_pool = ctx.enter_context(tc.tile_pool(name="pp", bufs=3))
        vpool = ctx.enter_context(tc.tile_pool(name="vpool", bufs=1))
        qk_pool = ctx.enter_context(tc.tile_pool(name="qkT", bufs=2))
        ctx_pool = ctx.enter_context(tc.tile_pool(name="ctxp", bufs=1))
        norm_pool = ctx.enter_context(tc.tile_pool(name="normp", bufs=3))
        dram_pool = ctx.enter_context(tc.tile_pool(name="dramp", bufs=2, space="DRAM"))
        psum_s = ctx.enter_context(tc.tile_pool(name="psum_s", bufs=2, space="PSUM"))
        psum_c = ctx.enter_context(tc.tile_pool(name="psum_c", bufs=2, space="PSUM"))
        psum_ctx = ctx.enter_context(tc.tile_pool(name="psum_ctx", bufs=2, space="PSUM"))

        # ---- constants / weights -------------------------------------------
        ones_row = consts.tile([1, 512], bf)
        nc.vector.memset(ones_row[:], 1.0)

        wq_sb = wpool.tile([128, NKD, DH], bf, tag="wq")
        wk_sb = wpool.tile([128, NKD, DH], bf, tag="wk")
        wv_sb = wpool.tile([128, NKD, DH], bf, tag="wv")
        bq_sb = bk_sb = bv_sb = None
        if has_bias:
            bq_sb = consts.tile([1, DH], bf, tag="bq")
            bk_sb = consts.tile([1, DH], bf, tag="bk")
            bv_sb = consts.tile([1, DH], bf, tag="bv")

        # load order: K-proj inputs first so PE starts earliest, then Q, V; wo last
        nc.sync.dma_start(out=wk_sb[:], in_=wk_d[:].rearrange("p (c n) -> p c n", c=NKD))
        xq_tiles, xk_tiles = [], []
        for k in range(NKD):
            tk = xqk_pool.tile([128, S], bf, name=f"xk{k}", tag=f"xk{k}")
            nc.sync.dma_start(out=tk[:], in_=xk_d[128 * k:128 * (k + 1), :])
            xk_tiles.append(tk)
        nc.sync.dma_start(out=wq_sb[:], in_=wq_d[:].rearrange("p (c n) -> p c n", c=NKD))
        if has_bias:
            nc.sync.dma_start(out=bq_sb[:], in_=bq_d[:])
            nc.sync.dma_start(out=bk_sb[:], in_=bk_d[:])
            nc.sync.dma_start(out=bv_sb[:], in_=bv_d[:])
        for k in range(NKD):
            tq = xqk_pool.tile([128, S], bf, name=f"xq{k}", tag=f"xq{k}")
            nc.sync.dma_start(out=tq[:], in_=xq_d[128 * k:128 * (k + 1), :])
            xq_tiles.append(tq)
        nc.sync.dma_start(out=wv_sb[:], in_=wv_d[:].rearrange("p (c n) -> p c n", c=NKD))
        xv_tiles = []
        for k in range(NKD):
            t = xv_pool.tile([128, S], bf, name=f"xv{k}", tag="xvt")
            nc.sync.dma_start(out=t[:], in_=xv_d[128 * k:128 * (k + 1), :])
            xv_tiles.append(t)

        v_sb = [None] * NSC

        def emit_v_chunk(s):
            ps = psum_c.tile([128, 512], f32, name=f"vps{s}", tag="ps512")
            for k in range(NKD):
                nc.tensor.matmul(
                    ps[:], xv_tiles[k][:, 128 * s:128 * (s + 1)], wv_sb[:, k, :],
                    start=(k == 0), stop=(not has_bias and k == NKD - 1))
            if has_bias:
                nc.tensor.matmul(ps[:], ones_row[0:1, 0:128], bv_sb[:],
                                 start=False, stop=True)
            vt = vpool.tile([128, H_CORE, DK + 1], bf, name=f"v{s}", tag=f"v{s}")
            nc.vector.tensor_copy(
                out=vt[:, :, 0:DK],
                in_=ps[:].rearrange("p (h d) -> p h d", h=H_CORE))
            nc.vector.memset(vt[:, :, DK:DK + 1], 1.0)
            v_sb[s] = vt

        def emit_proj_ntile(args):
            w_sb, b_sb, x_tiles, dst, hp, n = args
            ps = psum_c.tile([128, 512], f32, name=f"pj{hp}{n}", tag="ps512")
            for k in range(NKD):
                nc.tensor.matmul(
                    ps[:],
                    w_sb[:, k, 128 * hp:128 * (hp + 1)],
                    x_tiles[k][:, 512 * n:512 * (n + 1)],
                    start=(k == 0), stop=(not has_bias and k == NKD - 1))
            if has_bias:
                nc.tensor.matmul(
                    ps[:], b_sb[0:1, 128 * hp:128 * (hp + 1)],
                    ones_row[0:1, 0:512],
                    start=False, stop=True)
            nc.vector.tensor_copy(out=dst[:, 512 * n:512 * (n + 1)], in_=ps[:])

        ctx_sb = [ctx_pool.tile([128, S], bf, name=f"ctx{hp}", tag=f"ctx{hp}")
                  for hp in range(NHP)]

        # qT/kT tiles allocated one hp ahead (projections are pipelined into
        # the previous head pair's attention loop)
        qk_tiles = {}

        def get_qk(hp):
            if hp not in qk_tiles:
                qT = qk_pool.tile([128, S], bf, name=f"qT{hp}", tag="qT")
                kT = qk_pool.tile([128, S], bf, name=f"kT{hp}", tag="kT")
                qk_tiles[hp] = (qT, kT)
            return qk_tiles[hp]

        # ---- per head pair: attention (with pipelined proj of hp+1) --------
        for hp in range(NHP):
            qT, kT = get_qk(hp)
            if hp == 0:
                emit_proj_ntile((wk_sb, bk_sb, xk_tiles, kT, 0, 0))
            # work queue of next-hp projection tiles to sprinkle into qt2/qt3
            pending = []
            if hp + 1 < NHP:
                nqT, nkT = get_qk(hp + 1)
                for n in range(NQT):
                    pending.append((wk_sb, bk_sb, xk_tiles, nkT, hp + 1, n))
                for n in range(NQT):
                    pending.append((wq_sb, bq_sb, xq_tiles, nqT, hp + 1, n))

            cu_store = {}
            for qt in range(NQT):
                if hp == 0:
                    emit_proj_ntile((wq_sb, bq_sb, xq_tiles, qT, 0, qt))
                cps0 = psum_ctx.tile([128, 512], f32, name=f"c0_{hp}_{qt}", tag="cps")
                cps1 = psum_ctx.tile([128, 512], f32, name=f"c1_{hp}_{qt}", tag="cps")
                for kc in range(NSC):
                    if hp == 0 and qt == 0:
                        if kc % 4 == 3 and kc < NSC - 1:
                            emit_proj_ntile((wk_sb, bk_sb, xk_tiles, kT, 0,
                                             (kc + 1) // 4))
                        emit_v_chunk(kc)
                    elif pending and qt >= 2 and kc % 4 == 0:
                        emit_proj_ntile(pending.pop(0))
                    st = psum_s.tile([128, 1024], f32, name=f"st{hp}{qt}{kc}", tag="st")
                    for j in range(2):  # head within pair
                        r0 = 64 * j
                        nc.tensor.matmul(
                            st[:, 512 * j:512 * (j + 1)],
                            kT[r0:r0 + 64, 128 * kc:128 * (kc + 1)],
                            qT[r0:r0 + 64, 512 * qt:512 * (qt + 1)],
                            start=True, stop=True,
                            tile_position=(r0, 0))
                    pt = p_pool.tile([128, 1024], bf, name=f"p{hp}{qt}{kc}",
                                     tag="pp")
                    nc.scalar.activation(out=pt[:], in_=st[:],
                                         func=mybir.ActivationFunctionType.Exp,
                                         scale=0.125)
                    for j, cps in enumerate((cps0, cps1)):
                        nc.tensor.matmul(
                            cps[0:DK + 1, :],
                            v_sb[kc][:, 2 * hp + j, :],
                            pt[:, 512 * j:512 * (j + 1)],
                            start=(kc == 0), stop=(kc == NSC - 1))
                # evict unnormalized ctx + sums; stage sums at {0,32,64,96}
                stg_i = qt // 2
                if qt % 2 == 0:
                    cu_store[f"stage{stg_i}"] = norm_pool.tile(
                        [128, 512], f32, name=f"stage{hp}_{stg_i}",
                        tag="stage", bufs=2)
                    nc.vector.memset(cu_store[f"stage{stg_i}"][:], 1.0)
                stage = cu_store[f"stage{stg_i}"]
                for j, cps in enumerate((cps0, cps1)):
                    r = 2 * (qt % 2) + j
                    nc.vector.tensor_copy(
                        out=stage[32 * r:32 * r + 1, :],
                        in_=cps[DK:DK + 1, :])
                    cu = norm_pool.tile([DK, 512], bf,
                                        name=f"cu{hp}{qt}{j}", tag="cu", bufs=5)
                    nc.vector.tensor_copy(out=cu[:], in_=cps[0:DK, :])
                    cu_store[(qt, j)] = cu
                if qt % 2 == 1 or qt == NQT - 1:
                    # normalize this q-tile pair's (qt, j) rows
                    recips = norm_pool.tile([128, 512], f32,
                                            name=f"rc{hp}_{stg_i}",
                                            tag="recips", bufs=2)
                    nc.vector.reciprocal(out=recips[:], in_=stage[:])
                    rdram = dram_pool.tile([4, 512], f32,
                                           name=f"rd{hp}{stg_i}", tag="rd")
                    for r in range(4):
                        nc.sync.dma_start(
                            out=rdram[r:r + 1, :],
                            in_=recips[32 * r:32 * r + 1, :])
                    for qtt in ([qt] if qt % 2 == 0 else [qt - 1, qt]):
                        for j in range(2):
                            r = 2 * (qtt % 2) + j
                            bcast = norm_pool.tile(
                                [DK, 512], f32, name=f"b{hp}{qtt}{j}",
                                tag="bcast", bufs=2)
                            nc.sync.dma_start(
                                out=bcast[:],
                                in_=rdram[r:r + 1, :].to_broadcast([DK, 512]))
                            nc.vector.tensor_mul(
                                out=ctx_sb[hp][64 * j:64 * (j + 1),
                                               512 * qtt:512 * (qtt + 1)],
                                in0=cu_store[(qtt, j)][:], in1=bcast[:])
            # small-S configs: drain any proj work not absorbed by qt>=2 slots
            for args in pending:
                emit_proj_ntile(args)

        # ---- output projection (single pass, psum-accumulated) -------------
        wo_t = {}
        for kc in range(NKO):
            t = wpool.tile([128, D], bf, name=f"wo{kc}", tag=f"wo{kc}", bufs=1)
            nc.sync.dma_start(out=t[:], in_=wo_d[:, D * kc:D * (kc + 1)])
            wo_t[kc] = t
        pools = [(psum_c, "ps512"), (psum_s, "st"), (psum_ctx, "cps")]
        for mo in range(NMO):
            for nt in range(NQT):
                i_t = mo * NQT + nt
                pool, ptag = pools[i_t % 3]
                ps = pool.tile([128, 512], f32, name=f"ops{mo}{nt}", tag=ptag)
                for kc in range(NKO):
                    nc.tensor.matmul(
                        ps[:],
                        wo_t[kc][:, 128 * mo:128 * (mo + 1)],
                        ctx_sb[kc][:, 512 * nt:512 * (nt + 1)],
                        start=(kc == 0), stop=(kc == NKO - 1))
                # reuse dead xq slots as a deep output staging ring
                ot = xqk_pool.tile([128, 512], f32, name=f"o{mo}{nt}",
                                   tag=f"xq{i_t % 8}")
                if i_t % 2 == 0:
                    nc.vector.tensor_copy(out=ot[:], in_=ps[:])
                else:
                    nc.scalar.copy(out=ot[:], in_=ps[:])
                dst = out_d[128 * mo:128 * (mo + 1), 512 * nt:512 * (nt + 1)]
                nc.sync.dma_start(out=dst, in_=ot[:])


# ----------------------------------------------------------------------------
# host side
# ----------------------------------------------------------------------------

def _tile_w(wT, nchunks):
    """[K, M] -> [128, nchunks*M] partition-major bf16."""
    K, M = wT.shape
    assert K == nchunks * 128
    return np.ascontiguousarray(
        wT.reshape(nchunks, 128, M).transpose(1, 0, 2).reshape(128, nchunks * M)
    ).astype(BF16)


def make_in_maps(query, key, value, Wq, bq, Wk, bk, Wv, bv, Wo, S=2048):
    in_maps = []
    for c in range(N_CORES):
        b, g = divmod(c, 2)
        hd = slice(DH * g, DH * (g + 1))
        m = {
            "xq": np.ascontiguousarray(query[:, b, :].T).astype(BF16),
            "xk": np.ascontiguousarray(key[:, b, :].T).astype(BF16),
            "xv": np.ascontiguousarray(value[:, b, :].T).astype(BF16),
            "wq": _tile_w(np.ascontiguousarray(Wq[hd, :].T), D // 128),
            "wk": _tile_w(np.ascontiguousarray(Wk[hd, :].T), D // 128),
            "wv": _tile_w(np.ascontiguousarray(Wv[hd, :].T), D // 128),
            "wo": _tile_w(np.ascontiguousarray(Wo[:, hd].T), DH // 128),
            "bq": bq[hd].reshape(1, DH).astype(BF16),
            "bk": bk[hd].reshape(1, DH).astype(BF16),
            "bv": bv[hd].reshape(1, DH).astype(BF16),
        }
        in_maps.append(m)
    return in_maps


def combine_outputs(results, bo, S=2048):
    out = np.empty((S, 4, D), np.float32)
    for b in range(4):
        pT = results[2 * b]["out"] + results[2 * b + 1]["out"]  # [D, S]
        out[:, b, :] = pT.T
    out += bo.astype(np.float32)[None, None, :]
    return out


_NC_CACHE = {}


def get_program(S=2048, has_bias=True):
    key = (S, has_bias)
    if key not in _NC_CACHE:
        _NC_CACHE[key] = build_program(S, has_bias=has_bias)
    return _NC_CACHE[key]


def kernel(query, key, value, Wq, bq, Wk, bk, Wv, bv, Wo, bo, **_):
    query = np.asarray(query, np.float32)
    key = np.asarray(key, np.float32)
    value = np.asarray(value, np.float32)
    S = query.shape[0]
    has_bias = any(float(np.abs(np.asarray(b)).max()) != 0.0
                   for b in (bq, bk, bv))
    nc = get_program(S, has_bias=has_bias)
    in_maps = make_in_maps(query, key, value,
                           np.asarray(Wq), np.asarray(bq),
                           np.asarray(Wk), np.asarray(bk),
                           np.asarray(Wv), np.asarray(bv),
                           np.asarray(Wo), S=S)
    res = run_bass_kernel_spmd(nc, in_maps, core_ids=list(range(N_CORES)))
    return combine_outputs(res.results, np.asarray(bo), S=S)
